# revision 2
# baseline (speedup 1.0000x reference)
"""CRF NLL loss kernel v2 — segmented rank-1-stitched forward scan.

Math: Z_b = s^T A_{511}...A_1 v0 with A_t = D_{exp f_t} E^T (exp domain, E =
exp(transitions)). Products of positive matrices contract to rank-1 at
~e^-3/step (Birkhoff), so time splits into K=32 segments of L=16 steps.
Per segment k: fwd chain F_k = M_k @ init (init = ones, except seg 0 = v0)
and bwd chain B_k = M_k^T @ ones. The terminal s-dot folds into a dummy
512th step whose f-slice is onehot(STOP). Then
    logZ = sum_{g=0..30} log(B_{g+1}.F_g) - sum_{g=1..30} log(1^T F_g)
         + L*MU*K              (MU: e^-MU folded into the stationary W)
Numpy-validated: abs err ~0.02 per batch (rel ~1e-5) incl. bf16 state.

All 62 chains pack as 31 column groups x 8 batches: fwd_g in partitions
0-49, bwd_{g+1} in 64-113, block-diag stationary W = (E | E^T)*e^-MU.
2 streams (16+15 groups) of L=16 rounds; each round = 1 matmul + 2
tensor-tensor (DVE fwd half, gpsimd bwd half) with f-slices read straight
from the transposed exp-feats buffer via strided APs.

Data movement: host sends feats as bf16 [512, 128] per sequence with the
50 tags duplicated at cols 0:50 and 64:114 (gaps zero); ONE XBAR
transpose-DMA per sequence lands the full [tag, t] layout for both chain
halves including zeroed dead rows. exp runs as a few wide ACT ops.
Gold: emit = sum feats[tags] via TTR of the raw transposed feats with a
transposed one-hot (same XBAR path); transition-pair counts are pure
index work done on host (like prev/end prep), reduced against T on
device.
"""

import numpy as np

TAG = 50
START = TAG - 2
STOP = TAG - 1
B, S = 64, 512
NCORES = 8
BPC = B // NCORES
L = 8             # segment length (rounds per chain)
K = 64            # segments
G = K - 1         # column groups (fwd_g + bwd_{g+1}); 63
MU = 5.2          # growth prescale folded into W
NSLOT = 576       # ef slots per b (512 + 64 host-side aux rows)
STREAMS = [(0, 32), (32, 31)]

_COMPILED = {}
SKIP = set()
LAST_RESULTS = None
LAST_IN_MAPS = None


def _build():
    import concourse.bass as bass
    import concourse.bacc as bacc
    import concourse.tile as tile
    from concourse import mybir

    f32 = mybir.dt.float32
    bf16 = mybir.dt.bfloat16
    AF = mybir.ActivationFunctionType
    ALU = mybir.AluOpType
    AX = mybir.AxisListType

    nc = bacc.Bacc("TRN2", target_bir_lowering=False, debug=False,
                   enable_asserts=False, num_devices=NCORES)

    featsb = nc.dram_tensor("featsb", [BPC * NSLOT, 128], bf16,
                            kind="ExternalInput")
    ohtb = nc.dram_tensor("ohtb", [BPC * S, 128], bf16,
                          kind="ExternalInput")
    cmat = nc.dram_tensor("cmat", [TAG, TAG], f32, kind="ExternalInput")
    trans = nc.dram_tensor("trans", [TAG, TAG], f32, kind="ExternalInput")
    out = nc.dram_tensor("out", [1, 16], f32, kind="ExternalOutput")

    EMU = float(np.float32(np.exp(-MU)))

    with tile.TileContext(nc) as tc:
        with tc.tile_pool(name="const", bufs=1) as cpool, \
             tc.tile_pool(name="big", bufs=1) as bigpool, \
             tc.tile_pool(name="small", bufs=4) as spool, \
             tc.tile_pool(name="work", bufs=2) as wpool, \
             tc.tile_pool(name="x", bufs=3) as xpool, \
             tc.tile_pool(name="ps_tr", bufs=2, space="PSUM") as ps_tr, \
             tc.tile_pool(name="ps_s", bufs=4, space="PSUM") as ps_s, \
             tc.tile_pool(name="ps_m", bufs=2, space="PSUM") as ps_m:

            # ---------- constants ----------
            iota_col_i = cpool.tile([128, 1], mybir.dt.int32)
            nc.gpsimd.iota(iota_col_i[:], pattern=[[0, 1]], base=0,
                           channel_multiplier=1)
            iota_col_f = cpool.tile([128, 1], f32)
            nc.vector.tensor_copy(iota_col_f[:], iota_col_i[:])
            iota_row_i = cpool.tile([128, 128], mybir.dt.int32)
            nc.gpsimd.iota(iota_row_i[:], pattern=[[1, 128]], base=0,
                           channel_multiplier=0)
            iota_row_f = cpool.tile([128, 128], f32)
            nc.vector.tensor_copy(iota_row_f[:], iota_row_i[:])
            ident = cpool.tile([128, 128], f32)
            nc.vector.tensor_scalar(ident[:], iota_row_f[:], iota_col_f[:],
                                    None, op0=ALU.is_equal)
            ones50b = cpool.tile([TAG, 1], bf16)
            nc.vector.memset(ones50b[:], 1.0)
            ones50 = cpool.tile([TAG, 1], f32)
            nc.vector.memset(ones50[:], 1.0)

            # ---------- W stationary: block-diag (E, E^T) * e^-MU ----------
            tsb = cpool.tile([TAG, TAG], f32)
            nc.sync.dma_start(tsb[:], trans[:, :])
            expT0 = cpool.tile([TAG, TAG], f32)
            nc.scalar.activation(expT0[:], tsb[:], AF.Exp)
            expTs = cpool.tile([TAG, TAG], f32)   # E * e^-MU
            nc.vector.tensor_scalar(expTs[:], expT0[:], EMU, None,
                                    op0=ALU.mult)
            W = cpool.tile([128, 128], bf16)
            nc.vector.memset(W[:], 0.0)
            nc.vector.tensor_copy(W[0:TAG, 0:TAG], expTs[:])
            ttr_ps = ps_tr.tile([128, 128], f32, tag="tr")
            nc.tensor.transpose(ttr_ps[0:TAG, 0:TAG], expTs[:],
                                ident[0:TAG, 0:TAG])
            nc.vector.tensor_copy(W[64:64 + TAG, 64:64 + TAG],
                                  ttr_ps[0:TAG, 0:TAG])
            # E[START, :] as a [50,1] column via exp of T^T col START
            ttr2_ps = ps_tr.tile([128, 128], f32, tag="tr")
            nc.tensor.transpose(ttr2_ps[0:TAG, 0:TAG], tsb[:],
                                ident[0:TAG, 0:TAG])
            Tt = cpool.tile([TAG, TAG], f32)
            nc.vector.tensor_copy(Tt[:], ttr2_ps[0:TAG, 0:TAG])
            estart = cpool.tile([TAG, 1], f32)
            nc.scalar.activation(estart[:], Tt[:, START:START + 1], AF.Exp)

            # ---------- ef buffers, b-major: col = b*NSLOT + slot ----------
            efraw = bigpool.tile([128, BPC * NSLOT], bf16, name="efraw")
            efbuf = bigpool.tile([128, BPC * NSLOT], f32, name="efbuf")
            ohbuf = bigpool.tile([128, BPC * S], bf16, name="ohbuf")
            rawv = efraw[:].rearrange("p (e s) -> p e s", s=NSLOT)
            efv = efbuf[:].rearrange("p (e s) -> p e s", s=NSLOT)
            ohv = ohbuf[:].rearrange("p (e s) -> p e s", s=S)

            for b in range(0, BPC, 2):
                nc.sync.dma_start(
                    efraw[:, b * NSLOT:(b + 2) * NSLOT],
                    featsb[b * NSLOT:(b + 2) * NSLOT, :], transpose=True)
            for b in range(0, BPC, 2):
                nc.sync.dma_start(
                    ohbuf[:, b * S:(b + 2) * S],
                    ohtb[b * S:(b + 2) * S, :], transpose=True)

            # exp over everything incl. host aux rows (z0 / ones / dummy
            # are encoded by the host in log domain)
            for b in range(BPC):
                nc.scalar.activation(efv[:, b, :], rawv[:, b, :], AF.Exp)

            # ---------- gold ----------
            # emit = sum over (tag,t,b) of raw feats .* onehot (bf16 x bf16)
            emitc = bigpool.tile([TAG, BPC], f32, name="emitc")
            if "emit" in SKIP:
                nc.vector.memset(emitc[:], 0.0)
            else:
                for b in range(BPC):
                    em = wpool.tile([TAG, S], f32, tag="em")
                    te = nc.vector if b % 2 == 0 else nc.gpsimd
                    te.tensor_tensor(em[:], rawv[0:TAG, b, 0:S],
                                     ohv[0:TAG, b, :], op=ALU.mult)
                    nc.vector.tensor_reduce(emitc[:, b:b + 1], em[:],
                                            axis=AX.X, op=ALU.add)
            ep_ps = ps_m.tile([1, BPC], f32, tag="m")
            nc.tensor.matmul(ep_ps[:], ones50[:], emitc[:], start=True,
                             stop=True)
            gemit = spool.tile([1, 1], f32, tag="ge")
            nc.vector.tensor_reduce(gemit[:], ep_ps[:], axis=AX.X,
                                    op=ALU.add)
            # trans = sum(cmat .* T)   (cmat: host-side index histogram)
            csb = cpool.tile([TAG, TAG], f32)
            nc.sync.dma_start(csb[:], cmat[:, :])
            tm1 = wpool.tile([TAG, TAG], f32, tag="tm1")
            nc.vector.tensor_tensor(tm1[:], csb[:], tsb[:], op=ALU.mult)
            tred = spool.tile([TAG, 1], f32, tag="tred")
            nc.vector.tensor_reduce(tred[:], tm1[:], axis=AX.X, op=ALU.add)
            gt_ps = ps_m.tile([1, 1], f32, tag="m")
            nc.tensor.matmul(gt_ps[:], ones50[:], tred[:], start=True,
                             stop=True)
            gtrans = spool.tile([1, 1], f32, tag="gt")
            nc.vector.tensor_copy(gtrans[:], gt_ps[:])

            # ---------- scan ----------
            # efw view: [p, group, within, b];  slot = 16*group + within
            efw = efbuf[:].rearrange("p (e gg w) -> p gg w e",
                                     gg=NSLOT // L, w=L)
            streams = STREAMS
            xs = []
            for si, (g0, ng) in enumerate(streams):
                cols = ng * BPC
                X0 = xpool.tile([128, cols], bf16, tag=f"x{si}")
                nc.vector.memset(X0[0:64, :], 1.0)
                if si == 0 and "v0" not in SKIP:
                    # group 0 fwd init = v0 = ef[slot0] * E[START,:]
                    nc.vector.tensor_scalar(X0[0:TAG, 0:BPC],
                                            efv[0:TAG, :, 0], estart[:],
                                            None, op0=ALU.mult)
                # bwd init z0: host aux rows 513+g (upper half, exp'd)
                z0ap = efv[64:64 + TAG, :, 513 + g0:513 + g0 + ng]
                zeng = nc.gpsimd if si % 2 == 0 else nc.vector
                zeng.tensor_copy(
                    X0[64:64 + TAG, :],
                    z0ap.rearrange("p e g -> p g e"))
                xs.append(X0)

            for r in range(1, L + 1):
                for si, (g0, ng) in enumerate(streams):
                    cols = ng * BPC
                    X = xs[si]
                    ps = ps_s.tile([128, cols], f32, tag="s")
                    nc.tensor.matmul(ps[0:114, :], W[0:114, 0:114],
                                     X[0:114, :], start=True, stop=True)
                    X2 = xpool.tile([128, cols], bf16, tag=f"x{si}")
                    if r < L:
                        fsl = efw[0:114, g0:g0 + ng, r, :]
                    else:
                        fsl = efw[0:114, g0 + 1:g0 + 1 + ng, 0, :]
                    nc.vector.tensor_tensor(X2[0:114, :], ps[0:114, :],
                                            fsl, op=ALU.mult)
                    xs[si] = X2

            # ---------- stitch ----------
            Bst = bigpool.tile([TAG, G * BPC], bf16, name="bst")
            offs = [g0 * BPC for g0, _ in streams] + [G * BPC]
            for si, (g0, ng) in enumerate(streams):
                nc.sync.dma_start(Bst[:, offs[si]:offs[si + 1]],
                                  xs[si][64:64 + TAG, :])
            nrow_ps = ps_m.tile([1, G * BPC], f32, tag="m")
            for si, (g0, ng) in enumerate(streams):
                nc.tensor.matmul(nrow_ps[:, offs[si]:offs[si + 1]],
                                 ones50b[:], xs[si][0:TAG, :], start=True,
                                 stop=True)
            lnn = spool.tile([1, G * BPC], f32, tag="lnn")
            nc.scalar.activation(lnn[:], nrow_ps[:], AF.Ln)
            pq = bigpool.tile([TAG, G * BPC], bf16, name="pq")
            for si, (g0, ng) in enumerate(streams):
                eng = nc.vector if si % 2 == 0 else nc.gpsimd
                eng.tensor_tensor(pq[:, offs[si]:offs[si + 1]],
                                  xs[si][0:TAG, :],
                                  Bst[:, offs[si]:offs[si + 1]],
                                  op=ALU.mult)
            prow_ps = ps_m.tile([1, G * BPC], f32, tag="m")
            nc.tensor.matmul(prow_ps[:], ones50b[:], pq[:], start=True,
                             stop=True)
            lnp = spool.tile([1, G * BPC], f32, tag="lnp")
            nc.scalar.activation(lnp[:], prow_ps[:], AF.Ln)
            psum_b = spool.tile([1, BPC], f32, tag="psb")
            nc.vector.tensor_reduce(
                psum_b[:], lnp[:].rearrange("p (g e) -> p e g", e=BPC),
                axis=AX.X, op=ALU.add)
            nsum_b = spool.tile([1, BPC], f32, tag="nsb")
            nc.vector.tensor_reduce(
                nsum_b[:],
                lnn[:, BPC:].rearrange("p (g e) -> p e g", e=BPC),
                axis=AX.X, op=ALU.add)
            fwdrow = spool.tile([1, BPC], f32, tag="fwd")
            nc.vector.tensor_tensor(fwdrow[:], psum_b[:], nsum_b[:],
                                    op=ALU.subtract)

            # ---------- output ----------
            osb = cpool.tile([1, 16], f32, tag="osb")
            nc.vector.memset(osb[:], 0.0)
            nc.vector.tensor_copy(osb[:, 0:BPC], fwdrow[:])
            nc.vector.tensor_copy(osb[:, 8:9], gemit[:])
            nc.vector.tensor_copy(osb[:, 9:10], gtrans[:])
            nc.sync.dma_start(out[:, :], osb[:])

    nc.compile()
    return nc, "out"


def _numpy_reference(feats, mask, tags, transitions):
    maskf = mask.astype(np.float64)
    f = feats.astype(np.float64)
    T = transitions.astype(np.float64)
    b, s, t = f.shape
    part = f[:, 0, :] + T[START][None, :]
    for ti in range(1, s):
        cur = part[:, :, None] + T[None, :, :] + f[:, ti, None, :]
        m = cur.max(axis=1)
        cur = m + np.log(np.exp(cur - m[:, None, :]).sum(axis=1))
        part = np.where(mask[:, ti][:, None].astype(bool), cur, part)
    term = part[:, :, None] + T[None, :, :]
    m = term.max(axis=1)
    term = m + np.log(np.exp(term - m[:, None, :]).sum(axis=1))
    forward = term[:, STOP].sum()
    prev = np.concatenate([np.full((b, 1), START, dtype=tags.dtype),
                           tags[:, :-1]], axis=1)
    emit = np.take_along_axis(f, tags[..., None], axis=2)[..., 0]
    tr = T[prev, tags]
    tg = ((emit + tr) * maskf).sum()
    lengths = mask.astype(np.int64).sum(axis=1)
    end_ids = np.take_along_axis(tags, (lengths - 1)[:, None], axis=1)[:, 0]
    gold = tg + T[end_ids, STOP].sum()
    return np.array(forward - gold, dtype=np.float32)


def make_in_maps(feats, mask, tags, transitions):
    import ml_dtypes
    feats = np.asarray(feats, dtype=np.float32)
    tags_i = np.asarray(tags).astype(np.int64)
    mask_i = np.asarray(mask).astype(np.int64)
    transitions = np.asarray(transitions, dtype=np.float32)

    lengths = mask_i.sum(axis=1)
    end_ids = np.take_along_axis(tags_i, (lengths - 1)[:, None], axis=1)[:, 0]

    # feats, host-choreographed: [B, NSLOT, 128] bf16.
    # lower cols 0:50: rows t=0..511 = feats[t] (fwd sequence).
    # upper cols 64:114: row q = L*g+m (m=1..L-1) holds feats[L*g+2L-m]
    #   (the bwd-ordered sequence); rows L*k hold 0 (exp->1, the bwd
    #   final-round ones); aux rows 513+g hold the bwd z0 = feats[L*g+2L]
    #   for g<G-1, and the dummy onehot(STOP) in log domain for g=G-1.
    fb = np.zeros((B, NSLOT, 128), np.float32)
    fb[:, 0:S, 0:TAG] = feats
    q = np.arange(1, S)
    m = q % L
    src_t = L * (q // L) + 2 * L - m
    valid = (m != 0) & (src_t < S) & (q <= L * (G - 1) + L - 1)
    fb[:, q[valid], 64:64 + TAG] = feats[:, src_t[valid], :]
    g = np.arange(G - 1)
    fb[:, 513 + g, 64:64 + TAG] = feats[:, L * g + 2 * L, :]
    fb[:, 513 + (G - 1), 64:64 + TAG] = -100.0
    fb[:, 513 + (G - 1), 64 + STOP] = 0.0
    fb = fb.astype(ml_dtypes.bfloat16)
    fb = fb.reshape(B * NSLOT, 128).reshape(B, NSLOT, 128)
    # transposed-ready one-hot of tags (cols 0:50 only)
    oht = np.zeros((B, S, 128), np.float32)
    bb, tt = np.meshgrid(np.arange(B), np.arange(S), indexing="ij")
    oht[bb, tt, tags_i] = 1.0
    oht = oht.astype(ml_dtypes.bfloat16)
    # transition-pair histogram (pure index work), incl end->STOP
    prev = np.concatenate([np.full((B, 1), START, np.int64),
                           tags_i[:, :-1]], axis=1)
    cm = np.zeros((NCORES, TAG, TAG), np.float32)
    for c in range(NCORES):
        sl = slice(c * BPC, (c + 1) * BPC)
        np.add.at(cm[c], (prev[sl].ravel(), tags_i[sl].ravel()), 1.0)
        np.add.at(cm[c], (end_ids[sl], np.full(BPC, STOP)), 1.0)

    in_maps = []
    for c in range(NCORES):
        sl = slice(c * BPC, (c + 1) * BPC)
        in_maps.append({
            "featsb": np.ascontiguousarray(
                fb[sl].reshape(BPC * NSLOT, 128)),
            "ohtb": np.ascontiguousarray(
                oht[sl].reshape(BPC * S, 128)),
            "cmat": cm[c],
            "trans": transitions,
        })
    return in_maps


CONST_PER_BATCH = L * MU * K


def kernel(feats, mask, tags, transitions):
    global _COMPILED, LAST_RESULTS, LAST_IN_MAPS
    feats = np.asarray(feats, dtype=np.float32)
    mask = np.asarray(mask)
    tags = np.asarray(tags)
    transitions = np.asarray(transitions, dtype=np.float32)

    if not np.all(mask == 1):
        return _numpy_reference(feats, np.asarray(mask, dtype=np.int64),
                                np.asarray(tags, dtype=np.int64), transitions)

    if 1 not in _COMPILED:
        _COMPILED[1] = _build()
    nc, out_name = _COMPILED[1]

    in_maps = make_in_maps(feats, mask, tags, transitions)

    from concourse import bass_utils
    res = bass_utils.run_bass_kernel_spmd(nc, in_maps,
                                          core_ids=list(range(NCORES)))
    LAST_RESULTS = res
    LAST_IN_MAPS = in_maps

    total = 0.0
    for c in range(NCORES):
        o = res.results[c][out_name].astype(np.float64)[0]
        total += (o[0:BPC].sum() + BPC * CONST_PER_BATCH) - o[8] - o[9]
    return np.array(total, dtype=np.float32)
